# revision 1
# baseline (speedup 1.0000x reference)
"""Trainium2 Bass kernel for nn_Attention (B=8, N=1024, C=768, H=12).

Data-parallel over batch: core b handles batch element b.

Math (re-associated to avoid the huge bhqk,bhqd->bkd contraction):
  q = x Wq^T, k = x Wk^T             (per head h: qh, kh  [N, Z])
  S_h = qh kh^T * scale              [N, N]
  E_h = exp(S_h)   (scores are in [-3, 3]; no max-subtraction needed)
  den[qi] = sum_ki E_h[qi, ki]
  ks = kh / den[:, None], qs = qh / den[:, None]
  AT_h = [E_h^T ks ; E_h^T qs]^T     [2Z, N]   (A1T/A2T stacked)
  out  = sum_h AT_h^T @ M_hT + bp    with M_h = [Wq_h;Wk_h] @ Wp^T
         (head-combine and output projection fused on the host)

Structure:
  - natural-layout q/k (for the 1/den scaling) is NOT recomputed by
    matmul; qT/kT round-trip through DRAM and the DMA xbar transposes
    them into natkq[j] while the PE does real work.
  - phase B processes the 12 heads SERIALLY (one at_ps accumulator
    live at a time).  That frees PSUM for a 3-deep score-tile ring,
    which decouples the scores -> exp -> buffer-free latency chain
    that otherwise paces the kernel above the engine-throughput floor.
  - one exp tile per head-phase is computed on the Vector engine via a
    bf16 Schraudolph bit-trick (bitcast_int16(S*K1+K2)); its row-sum
    runs as a DVE reduce.  This sheds ~15% of the Scalar engine load,
    which is the phase-B throughput floor.
  - phase C: F[t] = sum_h AT_h[:,t]^T @ M_hT (fused combine+projection,
    96+96 MMs at the bf16 matmul roofline); bias is added by DVE during
    the PSUM->SBUF copy against a replicated [128, C] bias tile.
  - dummy matmuls warm the PE clock (HAM) during the input-DMA window
    and through the exp-paced final head-phases.

PSUM: psS pool 3 bufs x [128,1024] fp32 (6 banks) for scores /
projection chains / dummies / phase-C F tiles; psA pool 1 buf (2
banks) for the AT accumulator.  SBUF singles freed LIFO between
phases.
"""

import sys
from contextlib import ExitStack

import numpy as np

if "/opt/trn_rl_repo" not in sys.path:
    sys.path.insert(0, "/opt/trn_rl_repo")

import ml_dtypes
import concourse.bass as bass
import concourse.mybir as mybir
import concourse.tile as tile
from concourse import bacc, bass_utils
from concourse.bass import ts

B, N, C, H = 8, 1024, 768, 12
Z = C // H          # 64
P = 128
NT = N // P         # 8 qi tiles
CT = C // P         # 6 c tiles
SCALE = Z ** -0.5   # 0.125
FP = mybir.dt.float32
BF = mybir.dt.bfloat16
FPR = mybir.dt.float32r
I16 = mybir.dt.int16

CCH = [(0, 512), (512, 256)]  # C=768 split into matmul free-dim chunks

# Schraudolph bit-trick exp in bf16: bitcast_int16(round(s*K1 + K2)) is
# bf16(exp(s*SCALE)) with ~+-3% mantissa-interpolation ripple.  The
# ripple is common-mode between E and den (softmax ratio) and averages
# out over the 1024-term q-contraction, so end-to-end error stays
# ~1e-3.  Used to offload part of the exp work from the Scalar engine
# (the phase-B pacer) to the Vector engine.
EXP_K1 = SCALE * np.log2(np.e) * 128.0
EXP_K2 = 16256.0 - 0.0436 * 128.0

last_results = None  # set by kernel() for test harness introspection


def _r(ap):
    """bitcast to float32r for full-rate fp32 matmuls (fp32 data only)."""
    if ap.dtype == FP:
        return ap.bitcast(FPR)
    return ap


def emit(ctx: ExitStack, tc: tile.TileContext, io):
    nc = tc.nc
    xT, wqkT, M, bpr, out = io

    stack = []  # (name, free) in creation order; freed strictly LIFO

    def single(shape, dtype, name):
        t, free = tc.tile(shape, dtype, name=name)
        stack.append((name, free))
        return t

    def free_through(name):
        while stack:
            nm, fr = stack.pop()
            fr()
            if nm == name:
                return
        raise KeyError(name)

    # ---------------- PSUM pools: 3x2 + 1x2 = 8 banks -------------------
    psS = ctx.enter_context(tc.tile_pool(name="psS", bufs=3, space="PSUM"))
    psA = ctx.enter_context(tc.tile_pool(name="psA", bufs=1, space="PSUM"))

    def ps_tile():
        return psS.tile([P, N], FP, name="s", tag="s")

    # SBUF pools (entered before any single so LIFO holds at ctx exit)
    p_E = ctx.enter_context(tc.tile_pool(name="p_E", bufs=14))
    p_kqs = ctx.enter_context(tc.tile_pool(name="p_kqs", bufs=10))
    p_den = ctx.enter_context(tc.tile_pool(name="p_den", bufs=7))
    p_out = ctx.enter_context(tc.tile_pool(name="p_out", bufs=3))

    # ------------- singles, bottom of stack = longest-lived -------------
    M_all = single([P, H * C], BF, name="M_all")
    M_sb = [M_all[:, ts(h, C)] for h in range(H)]
    bp_sb = single([P, C], FP, name="bp_sb")
    AT_sb = [single([P, N], BF, name=f"AT{h}") for h in range(H)]
    # natkq[j]: [128, 2N] cols 0:N = k natural (t-major 128-col blocks),
    # N:2N = q natural; features c of heads 2j, 2j+1.
    natkq = [single([P, 2 * N], BF, name=f"natkq{j}") for j in range(CT)]
    # qT/kT tile j: [128, N] rows = c_out 128j..128j+127 (heads 2j, 2j+1)
    qT_sb = [single([P, N], BF, name=f"qT{j}") for j in range(CT)]
    kT_sb = [single([P, N], BF, name=f"kT{j}") for j in range(CT)]
    wqkT_all = single([P, CT * 2 * C], BF, name="wqkT_all")
    wqkT_sb = [wqkT_all[:, ts(i, 2 * C)] for i in range(CT)]
    xT_all = single([P, CT * N], BF, name="xT_all")
    xT_sb = [xT_all[:, ts(i, N)] for i in range(CT)]

    # DRAM scratch for the qT/kT -> natural-layout xbar transposes
    qkTd = []
    for j in range(CT):
        t_, _free = tc.tile([2, P, N], BF, space="DRAM", name=f"qkTd{j}")
        qkTd.append(t_)

    # HAM keep-warm scratch: the PE clock-gates to 1.2 GHz after ~3.4us
    # of low activity and needs ~3.4us of sustained work to recover;
    # dummy matmuls on a zeroed tile keep it at 2.4 GHz through the
    # input-DMA window and exp-paced stretches with no real PE work.
    warm_sb = single([P, 512], BF, name="warm_sb")
    nc.gpsimd.memset(warm_sb[:], 0)

    def dummy_mms(n):
        ps = ps_tile()
        for i in range(n):
            nc.tensor.matmul(ps[:, 0:512], lhsT=warm_sb[:, 0:P],
                             rhs=warm_sb[:], start=(i == 0), stop=(i == n - 1))

    # ---------------- batched input DMAs (phase-A inputs first) ---------
    for k in range(CT):
        nc.sync.dma_start(xT_sb[k][:], xT[ts(k, P), :])
        nc.sync.dma_start(wqkT_sb[k][:], wqkT[ts(k, P), :])
    # phase-C inputs follow on the same queue (needed only much later);
    # a second hwdge queue tangles the DMA semaphore ring and stalls the
    # input stream, so everything stays on sync.
    nc.sync.dma_start(M_all[:], M[:])
    nc.sync.dma_start(bp_sb[:], bpr[:])

    # ---------------- projection chains ----------------
    def chain(dst_ap, lhsT_of, rhs_of, width):
        """dst_ap = sum_k lhsT_of(k)^T @ rhs_of(k); psum chain + DVE copy."""
        ps = ps_tile()
        for k in range(CT):
            nc.tensor.matmul(
                ps[:, 0:width],
                lhsT=_r(lhsT_of(k)),
                rhs=_r(rhs_of(k)),
                start=(k == 0),
                stop=(k == CT - 1),
            )
        nc.vector.tensor_copy(dst_ap, ps[:, 0:width])

    def qkT_chains(j):
        # k chains + q-ch0 first: pair j's scores t=0..3 become ready one
        # chain earlier (they read kT fully but only qT cols 0:512).
        # One thunk per chain so callers can spread them across t-steps.
        def one(which, ch):
            cols = slice(512 * ch, 512 * ch + 512)
            dst = (qT_sb if which == 0 else kT_sb)[j][:, cols]
            woff = C * which
            chain(dst,
                  lambda k: wqkT_sb[k][:, woff + 128 * j: woff + 128 * j + P],
                  lambda k: xT_sb[k][:, cols], 512)
        return [lambda w=w, c=c: one(w, c) for w, c in
                [(1, 0), (0, 0), (1, 1), (0, 1)]]

    def emit_nat_dma(j):
        """qT/kT[j] -> DRAM -> xbar-transposed natural layout natkq[j]."""
        nc.sync.dma_start(qkTd[j][1], kT_sb[j][:])
        nc.sync.dma_start(qkTd[j][0], qT_sb[j][:])
        nc.sync.dma_start_transpose(
            natkq[j][:, 0:N].rearrange("p (t c) -> p t c", c=P),
            qkTd[j][1].rearrange("c (t q) -> c t q", q=P))
        nc.sync.dma_start_transpose(
            natkq[j][:, N:2 * N].rearrange("p (t c) -> p t c", c=P),
            qkTd[j][0].rearrange("c (t q) -> c t q", q=P))

    # warm the PE during the input-DMA window (no data dependencies), then
    # qT/kT for pair 0 up front so scores/exp start as early as possible
    for _ in range(3):
        dummy_mms(8)
    for th in qkT_chains(0):
        th()
    emit_nat_dma(0)

    # ---------------- phase B: 12 serial head-phases --------------------
    at_queue = []
    LAG = 8

    def drain_at(n):
        while len(at_queue) > n:
            at_queue.pop(0)()

    # extra work emitted inside each head-phase (fills exp-paced slack).
    # NB: trace order defines dependencies -- every producer must be
    # emitted before its first reader.  natkq[j]/qT/kT[j] chains+DMA for
    # pair j+1 are spread over pair j's two head-phases.
    def chain_frags(j, which, ch):
        # one projection chain split into two 3-matmul fragments that go
        # in adjacent extras slots: a whole 6-MM chain (~1.3us) inserted
        # into the Tensor queue ahead of a phase's ramping score matmuls
        # stalls the exp pipeline ~2.2us per pair seam; half-size
        # fragments fit the per-step slack.
        cols = slice(512 * ch, 512 * ch + 512)
        dst = (qT_sb if which == 0 else kT_sb)[j][:, cols]
        woff = C * which
        box = {}

        def f1():
            ps = ps_tile()
            box["ps"] = ps
            for k in range(3):
                nc.tensor.matmul(
                    ps[:, 0:512],
                    lhsT=wqkT_sb[k][:, woff + 128 * j: woff + 128 * j + P],
                    rhs=xT_sb[k][:, cols], start=(k == 0), stop=False)

        def f2():
            ps = box["ps"]
            for k in range(3, 6):
                nc.tensor.matmul(
                    ps[:, 0:512],
                    lhsT=wqkT_sb[k][:, woff + 128 * j: woff + 128 * j + P],
                    rhs=xT_sb[k][:, cols], start=False, stop=(k == 5))
            nc.vector.tensor_copy(dst, ps[:, 0:512])

        return f1, f2

    extras = {}
    for j in range(5):
        fr = []
        for which, ch in [(1, 0), (0, 0), (1, 1), (0, 1)]:
            fr.extend(chain_frags(j + 1, which, ch))
        extras[2 * j] = fr
        extras[2 * j + 1] = [lambda j=j: emit_nat_dma(j + 1)]
    # pair 5 gets no extras: in the serial-head structure its phases are
    # ~72% PE-busy with sub-us gaps, enough to hold HAM at 2.4 GHz
    # without dummy matmuls whose extra power draw feeds the P0 throttle

    for hp in range(H):
        j, par = hp // 2, hp & 1
        qt, kt = qT_sb[j], kT_sb[j]
        base = Z * par
        nat3 = natkq[j].rearrange("p (g t c) -> p g t c", g=2, c=P)
        den_t = p_den.tile([P, NT], FP, name="dent")
        rv_t = p_den.tile([P, NT], FP, name="rvt")
        at_ps = psA.tile([P, N], FP, name="at", tag="at")
        ext = list(extras.get(hp, []))
        for t in range(NT):
            S = ps_tile()
            for ch in range(2):
                cols = slice(512 * ch, 512 * ch + 512)
                nc.tensor.matmul(
                    S[:, cols],
                    lhsT=qt[base:base + Z, ts(t, P)],
                    rhs=kt[base:base + Z, cols],
                    start=True, stop=True,
                )
            if True:
                E = p_E.tile([P, N], BF, name="Et")
                if hp >= 2 and t in (2, 5):
                    # bit-trick exp + row-sum on the Vector engine
                    nc.vector.tensor_scalar(
                        E[:].bitcast(I16), S[:], EXP_K1, EXP_K2,
                        op0=mybir.AluOpType.mult, op1=mybir.AluOpType.add)
                    nc.vector.tensor_reduce(
                        den_t[:, t:t + 1], E[:],
                        axis=mybir.AxisListType.X, op=mybir.AluOpType.add)
                else:
                    nc.scalar.activation(
                        E[:], S[:], mybir.ActivationFunctionType.Exp,
                        scale=SCALE, accum_out=den_t[:, t:t + 1],
                    )

                def at_mm(t=t, E=E, at_ps=at_ps, rv_t=rv_t, nat3=nat3,
                          par=par):
                    kqs = p_kqs.tile([P, 2 * Z], BF, name="kqst")
                    nc.vector.tensor_scalar_mul(
                        kqs[:].rearrange("p (g z) -> p g z", g=2),
                        nat3[:, :, t, ts(par, Z)],
                        rv_t[:, t:t + 1],
                    )
                    for ch in range(2):
                        cols = slice(512 * ch, 512 * ch + 512)
                        nc.tensor.matmul(
                            at_ps[:, cols],
                            lhsT=kqs[:],
                            rhs=E[:, cols],
                            start=(t == 0), stop=(t == NT - 1),
                        )

                at_queue.append(at_mm)
                drain_at(LAG)
                if t == NT - 1:
                    # one batched reciprocal per head-phase; the
                    # LAG-deferred at_mm scale ops run next phase
                    nc.vector.reciprocal(rv_t[:], den_t[:])
                if ext:
                    th = ext.pop(0)
                    if th is not None:
                        th()

        def at_copy(hp=hp, at_ps=at_ps):
            # on the Scalar engine: ACT has slack and this keeps the DVE
            # free for the kqs/exp ops that pace the head-phase seams
            nc.scalar.copy(AT_sb[hp][:], at_ps[:])
        at_queue.append(at_copy)
    drain_at(0)

    free_through("natkq0")  # frees xT, wqkT, kT*, qT*, natkq*, warm_sb

    # ---------------- phase C: fused combine + projection + bias ------
    for t in range(NT):
        F_ps = ps_tile()
        for h in range(H):
            for off, w in CCH:
                nc.tensor.matmul(
                    F_ps[:, off:off + w],
                    lhsT=AT_sb[h][:, ts(t, P)],
                    rhs=M_sb[h][:, off:off + w],
                    start=(h == 0), stop=(h == H - 1),
                )
        o = p_out.tile([P, C], FP, name="outt")
        nc.vector.tensor_add(o[:], F_ps[:, 0:C], bp_sb[:])
        nc.sync.dma_start(out[ts(t, P), :], o[:])

    while stack:
        stack.pop()[1]()


def build():
    nc = bacc.Bacc("TRN2", target_bir_lowering=False, debug=False, num_devices=B)
    xT = nc.dram_tensor("xT", [C, N], BF, kind="ExternalInput").ap()
    wqkT = nc.dram_tensor("wqkT", [C, 2 * C], BF, kind="ExternalInput").ap()
    M = nc.dram_tensor("M", [P, H * C], BF, kind="ExternalInput").ap()
    bpr = nc.dram_tensor("bpr", [P, C], FP, kind="ExternalInput").ap()
    out = nc.dram_tensor("out", [N, C], FP, kind="ExternalOutput").ap()
    with tile.TileContext(nc) as tc, ExitStack() as ctx:
        emit(ctx, tc, (xT, wqkT, M, bpr, out))
    nc.compile()
    return nc


def kernel(x, Wq, Wk, Wp, bp, trace=False, **trace_kwargs):
    global last_results
    x = np.asarray(x, dtype=np.float32)
    Wq = np.asarray(Wq, dtype=np.float32)
    Wk = np.asarray(Wk, dtype=np.float32)
    Wp = np.asarray(Wp, dtype=np.float32)
    bp = np.asarray(bp, dtype=np.float32)

    nc = build()
    bf = ml_dtypes.bfloat16
    wqkTc = np.ascontiguousarray(
        np.concatenate([Wq.T, Wk.T], axis=1)).astype(bf)  # [C, 2C]
    # fused combine+projection weights: M_hT = [Wq_h; Wk_h] @ Wp^T  [2Z, C]
    Wq_h = Wq.reshape(H, Z, C)
    Wk_h = Wk.reshape(H, Z, C)
    W2 = np.concatenate([Wq_h, Wk_h], axis=1)             # [H, 2Z, C]
    M_np = np.einsum("hzc,dc->hzd", W2, Wp)               # [H, 2Z, C]
    Mc = np.ascontiguousarray(
        M_np.transpose(1, 0, 2).reshape(P, H * C)).astype(bf)
    bprc = np.ascontiguousarray(
        np.broadcast_to(bp.reshape(1, C), (P, C)).astype(np.float32))
    in_maps = []
    for b in range(B):
        in_maps.append({
            "xT": np.ascontiguousarray(x[b].T).astype(bf),
            "wqkT": wqkTc, "M": Mc, "bpr": bprc,
        })
    res = bass_utils.run_bass_kernel_spmd(
        nc, in_maps, core_ids=list(range(B)), trace=trace, **trace_kwargs)
    last_results = res
    return np.stack([res.results[b]["out"] for b in range(B)], axis=0)



# revision 3
# speedup vs baseline: 1.0119x; 1.0119x over previous
"""Trainium2 Bass kernel for nn_Attention (B=8, N=1024, C=768, H=12).

Data-parallel over batch: core b handles batch element b.

Math (re-associated to avoid the huge bhqk,bhqd->bkd contraction):
  q = x Wq^T, k = x Wk^T             (per head h: qh, kh  [N, Z])
  S_h = qh kh^T * scale              [N, N]
  E_h = exp(S_h)   (scores are in [-3, 3]; no max-subtraction needed)
  den[qi] = sum_ki E_h[qi, ki]
  ks = kh / den[:, None], qs = qh / den[:, None]
  AT_h = [E_h^T ks ; E_h^T qs]^T     [2Z, N]   (A1T/A2T stacked)
  out  = sum_h AT_h^T @ M_hT + bp    with M_h = [Wq_h;Wk_h] @ Wp^T

Round-1 restructure vs baseline:
  - phase B processes head PAIRS (2j, 2j+1): their score matmuls have
    K=Z=64 contraction and live in disjoint PE row groups (rows 0-63 /
    64-127, via base_partition-derived tile_position), so interleaving
    the two heads' score MMs runs them CONCURRENTLY in the PE array --
    ~2x score throughput vs the serial-head structure.
  - E tiles are fp8e4m3, written pairwise into [128, 2, N] tiles
    (planes = q-tiles 2u, 2u+1).  The AT accumulation runs as fp8
    DoubleRow matmuls (contraction 256 = 2 q-tiles per MM) -- half the
    AT matmul count at +13%/MM.
  - kqs = nat(k,q) * (2^11/den) in fp8e4m3 (2^11 rescue from fp8
    underflow; compensated by M * 2^-11 on the host).
  - exp split: most tiles on ACT (fused exp+den via accum_out), a few
    per pair on DVE via an int8 Schraudolph bit-trick that emits
    fp8e4m3 bits directly, + a DVE row-sum for den.
"""

import sys
from contextlib import ExitStack

import numpy as np

if "/opt/trn_rl_repo" not in sys.path:
    sys.path.insert(0, "/opt/trn_rl_repo")

import ml_dtypes
import concourse.bass as bass
import concourse.mybir as mybir
import concourse.tile as tile
from concourse import bacc, bass_utils
from concourse.bass import ts

B, N, C, H = 8, 1024, 768, 12
Z = C // H          # 64
P = 128
NT = N // P         # 8 qi tiles
CT = C // P         # 6 c tiles
NP = H // 2         # 6 head pairs
SCALE = Z ** -0.5   # 0.125
FP = mybir.dt.float32
BF = mybir.dt.bfloat16
F8 = mybir.dt.float8e4
I8 = mybir.dt.int8
KQS_SHIFT = 11      # kqs scaled by 2^11 (fp8 range); M * 2^-11 on host
DR = mybir.MatmulPerfMode.DoubleRow

CCH = [(0, 512), (512, 256)]  # C=768 split into matmul free-dim chunks

# Schraudolph bit-trick exp emitting fp8e4m3 bits via int8 convert:
# bitcast_int8(round(s*K1_8 + K2_8)) ~= fp8e4m3(exp(s*SCALE)).
# e4m3: 3 mantissa bits (factor 8), exponent bias 7.
EXP_K1_8 = SCALE * np.log2(np.e) * 8.0
EXP_K2_8 = 7.0 * 8.0 - 0.0436 * 8.0

# (parity, t) tiles handled by the DVE bit-trick (pairs >= 1)
DVE_SET = {(0, 2), (1, 2), (0, 5), (1, 5)}

last_results = None  # set by kernel() for test harness introspection


def emit(ctx: ExitStack, tc: tile.TileContext, io):
    nc = tc.nc
    xT, wqkT, M, bpr, out = io

    stack = []  # (name, free) in creation order; freed strictly LIFO

    def single(shape, dtype, name):
        t, free = tc.tile(shape, dtype, name=name)
        stack.append((name, free))
        return t

    def free_through(name):
        while stack:
            nm, fr = stack.pop()
            fr()
            if nm == name:
                return
        raise KeyError(name)

    # ---------------- PSUM pools: 3x2 + 1x2 = 8 banks -------------------
    psS = ctx.enter_context(tc.tile_pool(name="psS", bufs=3, space="PSUM"))
    psA = ctx.enter_context(tc.tile_pool(name="psA", bufs=1, space="PSUM"))

    def ps_tile():
        return psS.tile([P, N], FP, name="s", tag="s")

    # SBUF pools (entered before any single so LIFO holds at ctx exit)
    p_E = ctx.enter_context(tc.tile_pool(name="p_E", bufs=11))
    p_kqs = ctx.enter_context(tc.tile_pool(name="p_kqs", bufs=6))
    p_den = ctx.enter_context(tc.tile_pool(name="p_den", bufs=8))
    p_out = ctx.enter_context(tc.tile_pool(name="p_out", bufs=3))

    # ------------- singles, bottom of stack = longest-lived -------------
    M_all = single([P, H * C], BF, name="M_all")
    M_sb = [M_all[:, ts(h, C)] for h in range(H)]
    bp_sb = single([P, C], FP, name="bp_sb")
    AT_sb = [single([P, N], BF, name=f"AT{h}") for h in range(H)]
    # natkq[j]: [128, 2N] cols 0:N = k natural (t-major 128-col blocks),
    # N:2N = q natural; features c of heads 2j, 2j+1.
    natkq = [single([P, 2 * N], BF, name=f"natkq{j}") for j in range(CT)]
    # qT/kT tile j: [128, N] rows = c_out 128j..128j+127 (heads 2j, 2j+1)
    qT_sb = [single([P, N], BF, name=f"qT{j}") for j in range(CT)]
    kT_sb = [single([P, N], BF, name=f"kT{j}") for j in range(CT)]
    wqkT_all = single([P, CT * 2 * C], BF, name="wqkT_all")
    wqkT_sb = [wqkT_all[:, ts(i, 2 * C)] for i in range(CT)]
    xT_all = single([P, CT * N], BF, name="xT_all")
    xT_sb = [xT_all[:, ts(i, N)] for i in range(CT)]

    # DRAM scratch for the qT/kT -> natural-layout xbar transposes
    qkTd = []
    for j in range(CT):
        t_, _free = tc.tile([2, P, N], BF, space="DRAM", name=f"qkTd{j}")
        qkTd.append(t_)

    # HAM keep-warm scratch: dummy matmuls hold the PE at 2.4 GHz
    # through the input-DMA window.
    warm_sb = single([P, 512], BF, name="warm_sb")
    nc.gpsimd.memset(warm_sb[:], 0)

    def dummy_mms(n):
        ps = ps_tile()
        for i in range(n):
            nc.tensor.matmul(ps[:, 0:512], lhsT=warm_sb[:, 0:P],
                             rhs=warm_sb[:], start=(i == 0), stop=(i == n - 1))

    # ---------------- batched input DMAs (phase-A inputs first) ---------
    for k in range(CT):
        nc.sync.dma_start(xT_sb[k][:], xT[ts(k, P), :])
        nc.sync.dma_start(wqkT_sb[k][:], wqkT[ts(k, P), :])
    nc.sync.dma_start(M_all[:], M[:])
    nc.sync.dma_start(bp_sb[:], bpr[:])

    # ---------------- projection chains ----------------
    def chain(dst_ap, lhsT_of, rhs_of, width):
        ps = ps_tile()
        for k in range(CT):
            nc.tensor.matmul(
                ps[:, 0:width],
                lhsT=lhsT_of(k),
                rhs=rhs_of(k),
                start=(k == 0),
                stop=(k == CT - 1),
            )
        nc.vector.tensor_copy(dst_ap, ps[:, 0:width])

    def qkT_chains(j):
        def one(which, ch):
            cols = slice(512 * ch, 512 * ch + 512)
            dst = (qT_sb if which == 0 else kT_sb)[j][:, cols]
            woff = C * which
            chain(dst,
                  lambda k: wqkT_sb[k][:, woff + 128 * j: woff + 128 * j + P],
                  lambda k: xT_sb[k][:, cols], 512)
        return [lambda w=w, c=c: one(w, c) for w, c in
                [(1, 0), (0, 0), (1, 1), (0, 1)]]

    def emit_nat_dma(j):
        """qT/kT[j] -> DRAM -> xbar-transposed natural layout natkq[j]."""
        nc.sync.dma_start(qkTd[j][1], kT_sb[j][:])
        nc.sync.dma_start(qkTd[j][0], qT_sb[j][:])
        nc.sync.dma_start_transpose(
            natkq[j][:, 0:N].rearrange("p (t c) -> p t c", c=P),
            qkTd[j][1].rearrange("c (t q) -> c t q", q=P))
        nc.sync.dma_start_transpose(
            natkq[j][:, N:2 * N].rearrange("p (t c) -> p t c", c=P),
            qkTd[j][0].rearrange("c (t q) -> c t q", q=P))

    # warm the PE during the input-DMA window, then qT/kT for pair 0 up
    # front so scores/exp start as early as possible
    for _ in range(3):
        dummy_mms(8)
    for th in qkT_chains(0):
        th()
    emit_nat_dma(0)

    # ---------------- phase B: 6 pair-phases ----------------------------
    at_queue = []

    def drain_at(n):
        while len(at_queue) > n:
            at_queue.pop(0)()

    # chain fragments for pair j+1, spread across pair j's t-steps
    def chain_frags(j, which, ch):
        cols = slice(512 * ch, 512 * ch + 512)
        dst = (qT_sb if which == 0 else kT_sb)[j][:, cols]
        woff = C * which
        box = {}

        def f1():
            ps = ps_tile()
            box["ps"] = ps
            for k in range(3):
                nc.tensor.matmul(
                    ps[:, 0:512],
                    lhsT=wqkT_sb[k][:, woff + 128 * j: woff + 128 * j + P],
                    rhs=xT_sb[k][:, cols], start=(k == 0), stop=False)

        def f2():
            ps = box["ps"]
            for k in range(3, 6):
                nc.tensor.matmul(
                    ps[:, 0:512],
                    lhsT=wqkT_sb[k][:, woff + 128 * j: woff + 128 * j + P],
                    rhs=xT_sb[k][:, cols], start=False, stop=(k == 5))
            nc.vector.tensor_copy(dst, ps[:, 0:512])

        return f1, f2

    for pj in range(NP):
        he, ho = 2 * pj, 2 * pj + 1
        qt, kt = qT_sb[pj], kT_sb[pj]
        nat3 = natkq[pj].rearrange("p (g t c) -> p g t c", g=2, c=P)
        den = [p_den.tile([P, NT], FP, name=f"den{par}") for par in range(2)]
        rv = [p_den.tile([P, NT], FP, name=f"rv{par}") for par in range(2)]
        # E pair-tiles per (parity, u): plane i holds q-tile t=2u+i
        Ep = [[None] * (NT // 2) for _ in range(2)]

        # extras: proj chains + nat DMA for pair pj+1
        ext = []
        if pj < NP - 1:
            for which, ch in [(1, 0), (0, 0), (1, 1), (0, 1)]:
                ext.extend(chain_frags(pj + 1, which, ch))
            ext.append(lambda j=pj: emit_nat_dma(j + 1))

        for t in range(NT):
            u, i = t // 2, t & 1
            S2 = []
            for par in range(2):
                S = ps_tile()
                S2.append(S)
                if i == 0:
                    Ep[par][u] = p_E.tile([P, 2, N], F8, name="Ep")
            # interleaved score MMs: the two heads target PE row groups
            # (0,0) / (64,0) (auto-derived from base_partition) and run
            # concurrently in the array
            for ch in range(2):
                cols = slice(512 * ch, 512 * ch + 512)
                for par in range(2):
                    base = Z * par
                    nc.tensor.matmul(
                        S2[par][:, cols],
                        lhsT=qt[base:base + Z, ts(t, P)],
                        rhs=kt[base:base + Z, cols],
                        start=True, stop=True,
                    )
            for par in range(2):
                E_ap = Ep[par][u][:, i, :]
                if pj >= 1 and (par, t) in DVE_SET:
                    # int8 Schraudolph -> fp8e4m3 bits, + DVE row-sum
                    nc.vector.tensor_scalar(
                        E_ap.bitcast(I8), S2[par][:], EXP_K1_8, EXP_K2_8,
                        op0=mybir.AluOpType.mult, op1=mybir.AluOpType.add)
                    nc.vector.tensor_reduce(
                        den[par][:, t:t + 1], E_ap,
                        axis=mybir.AxisListType.X, op=mybir.AluOpType.add)
                else:
                    nc.scalar.activation(
                        E_ap, S2[par][:], mybir.ActivationFunctionType.Exp,
                        scale=SCALE, accum_out=den[par][:, t:t + 1],
                    )
            # drain deferred AT work from the previous pair: head-e at
            # t=2, head-o at t=5 (each closure emits a full head's kqs +
            # DR matmuls; the Tile scheduler pipelines them into the
            # exp-paced slack)
            if t == 2:
                drain_at(1)
            elif t == 5:
                drain_at(0)
            # two extras slots per t-step (proj chains / nat DMA for the
            # next pair)
            for _ in range(2):
                if ext:
                    ext.pop(0)()

        # pair end: reciprocals, then queue this pair's AT work (the
        # 2^11 fp8-range rescue rides the kqs tensor_scalar's op1 slot)
        for par in range(2):
            nc.vector.reciprocal(rv[par][:], den[par][:])

        def at_head(par, pj=pj, nat3=nat3, Ep=Ep, rv=rv):
            h = 2 * pj + par

            def work():
                at_ps = psA.tile([P, N], FP, name="at", tag="at")
                for u in range(NT // 2):
                    kqs = p_kqs.tile([P, 2, 2 * Z], F8, name="kqst")
                    for i in range(2):
                        t = 2 * u + i
                        nc.vector.tensor_scalar(
                            kqs[:, i, :].rearrange("p (g z) -> p g z", g=2),
                            nat3[:, :, t, ts(par, Z)],
                            rv[par][:, t:t + 1], float(1 << KQS_SHIFT),
                            op0=mybir.AluOpType.mult,
                            op1=mybir.AluOpType.mult)
                    for ch in range(2):
                        cols = slice(512 * ch, 512 * ch + 512)
                        nc.tensor.matmul(
                            at_ps[:, cols],
                            lhsT=kqs[:, 0:2, :],
                            rhs=Ep[par][u][:, 0:2, cols],
                            start=(u == 0), stop=(u == NT // 2 - 1),
                            perf_mode=DR,
                        )
                nc.scalar.copy(AT_sb[h][:], at_ps[:])
            return work

        for par in range(2):
            at_queue.append(at_head(par))
    drain_at(0)

    free_through("natkq0")  # frees xT, wqkT, kT*, qT*, natkq*, warm_sb

    # ---------------- phase C: fused combine + projection + bias ------
    for t in range(NT):
        F_ps = ps_tile()
        for h in range(H):
            for off, w in CCH:
                nc.tensor.matmul(
                    F_ps[:, off:off + w],
                    lhsT=AT_sb[h][:, ts(t, P)],
                    rhs=M_sb[h][:, off:off + w],
                    start=(h == 0), stop=(h == H - 1),
                )
        o = p_out.tile([P, C], FP, name="outt")
        nc.vector.tensor_add(o[:], F_ps[:, 0:C], bp_sb[:])
        nc.sync.dma_start(out[ts(t, P), :], o[:])

    while stack:
        stack.pop()[1]()


def build():
    nc = bacc.Bacc("TRN2", target_bir_lowering=False, debug=False, num_devices=B)
    xT = nc.dram_tensor("xT", [C, N], BF, kind="ExternalInput").ap()
    wqkT = nc.dram_tensor("wqkT", [C, 2 * C], BF, kind="ExternalInput").ap()
    M = nc.dram_tensor("M", [P, H * C], BF, kind="ExternalInput").ap()
    bpr = nc.dram_tensor("bpr", [P, C], FP, kind="ExternalInput").ap()
    out = nc.dram_tensor("out", [N, C], FP, kind="ExternalOutput").ap()
    with tile.TileContext(nc) as tc, ExitStack() as ctx:
        emit(ctx, tc, (xT, wqkT, M, bpr, out))
    nc.compile()
    return nc


def kernel(x, Wq, Wk, Wp, bp, trace=False, **trace_kwargs):
    global last_results
    x = np.asarray(x, dtype=np.float32)
    Wq = np.asarray(Wq, dtype=np.float32)
    Wk = np.asarray(Wk, dtype=np.float32)
    Wp = np.asarray(Wp, dtype=np.float32)
    bp = np.asarray(bp, dtype=np.float32)

    nc = build()
    bf = ml_dtypes.bfloat16
    wqkTc = np.ascontiguousarray(
        np.concatenate([Wq.T, Wk.T], axis=1)).astype(bf)  # [C, 2C]
    # fused combine+projection weights: M_hT = [Wq_h; Wk_h] @ Wp^T  [2Z, C]
    # scaled by 2^-KQS_SHIFT to compensate the on-device kqs upscale
    Wq_h = Wq.reshape(H, Z, C)
    Wk_h = Wk.reshape(H, Z, C)
    W2 = np.concatenate([Wq_h, Wk_h], axis=1)             # [H, 2Z, C]
    M_np = np.einsum("hzc,dc->hzd", W2, Wp) * (2.0 ** -KQS_SHIFT)
    Mc = np.ascontiguousarray(
        M_np.transpose(1, 0, 2).reshape(P, H * C)).astype(bf)
    bprc = np.ascontiguousarray(
        np.broadcast_to(bp.reshape(1, C), (P, C)).astype(np.float32))
    in_maps = []
    for b in range(B):
        in_maps.append({
            "xT": np.ascontiguousarray(x[b].T).astype(bf),
            "wqkT": wqkTc, "M": Mc, "bpr": bprc,
        })
    res = bass_utils.run_bass_kernel_spmd(
        nc, in_maps, core_ids=list(range(B)), trace=trace, **trace_kwargs)
    last_results = res
    return np.stack([res.results[b]["out"] for b in range(B)], axis=0)


# revision 5
# speedup vs baseline: 1.0419x; 1.0296x over previous
"""Trainium2 Bass kernel for nn_Attention (B=8, N=1024, C=768, H=12).

Data-parallel over batch: core b handles batch element b.

Math (re-associated to avoid the huge bhqk,bhqd->bkd contraction):
  q = x Wq^T, k = x Wk^T             (per head h: qh, kh  [N, Z])
  S_h = qh kh^T * scale              [N, N]
  E_h = exp(S_h), den = rowsum(E_h)
  AT_h = [E^T (k/den) ; E^T (q/den)]^T   [2Z, N]
  out  = sum_h AT_h^T @ M_hT + bp    with M_h = [Wq_h;Wk_h] @ Wp^T

Structure (round 2'):
  - 6 head-PAIR phases; the two heads' score matmuls (K=Z=64) live in
    disjoint PE row groups (rows 0-63 / 64-127 via base_partition) and
    interleave -> concurrent in the PE array (~2x score throughput).
  - AT accumulation in fp8e4m3 DoubleRow (contraction 256 = 2 q-tiles
    per MM): E written as fp8 by the ACT exp, kqs = knat*rv*2^15 in fp8
    (2^15 compensated by a 2^-15 scale on the at_ps -> AT_sb copy).
    Projections and phase C stay bf16: their errors feed the output
    linearly (no softmax averaging) and fp8 there blows the error
    budget (verified against a numpy model of the full pipeline).
  - exp split: ACT handles 12/16 tiles per pair (fused exp+den via
    accum_out); the DVE handles u-groups (par0,u1) t=2,3 and (par1,u2)
    t=4,5 via a bf16 Schraudolph bit-trick + row-sum, consumed by bf16
    (non-DR) AT matmuls.  The split staggers ACT/DVE within a t-step.
  - AT work for pair j drains granularly (one u-group closure per
    t-step) through pair j+1; the last pair uses per-u reciprocals and
    drains its own AT work immediately to shorten the tail.
"""

import sys
from contextlib import ExitStack

import numpy as np

if "/opt/trn_rl_repo" not in sys.path:
    sys.path.insert(0, "/opt/trn_rl_repo")

import ml_dtypes
import concourse.bass as bass
import concourse.mybir as mybir
import concourse.tile as tile
from concourse import bacc, bass_utils
from concourse.bass import ts

B, N, C, H = 8, 1024, 768, 12
Z = C // H          # 64
P = 128
NT = N // P         # 8 qi tiles
CT = C // P         # 6 c tiles
NP = H // 2         # 6 head pairs
NU = NT // 2        # 4 q-tile pairs (DoubleRow u-groups)
SCALE = Z ** -0.5   # 0.125
FP = mybir.dt.float32
BF = mybir.dt.bfloat16
F8 = mybir.dt.float8e4
I16 = mybir.dt.int16
DR = mybir.MatmulPerfMode.DoubleRow

KQS_SH = 15         # kqs = knat * rv * 2^15 (fp8 range); at_copy * 2^-15
CCH = [(0, 512), (512, 256)]  # C=768 split into matmul free-dim chunks

# bf16 Schraudolph bit-trick exp for the DVE tiles
EXP_K1 = SCALE * np.log2(np.e) * 128.0
EXP_K2 = 16256.0 - 0.0436 * 128.0

# (parity, u) q-tile pairs handled by the DVE (bf16 E, non-DR AT)
DVE_U = {(0, 1), (1, 2)}

last_results = None  # set by kernel() for test harness introspection


def emit(ctx: ExitStack, tc: tile.TileContext, io):
    nc = tc.nc
    xT, wqkT, M, bpr, out = io

    stack = []  # (name, free) in creation order; freed strictly LIFO

    def single(shape, dtype, name):
        t, free = tc.tile(shape, dtype, name=name)
        stack.append((name, free))
        return t

    def free_through(name):
        while stack:
            nm, fr = stack.pop()
            fr()
            if nm == name:
                return
        raise KeyError(name)

    # ---------------- PSUM pools: 3x2 + 1x2 = 8 banks -------------------
    psS = ctx.enter_context(tc.tile_pool(name="psS", bufs=3, space="PSUM"))
    psA = ctx.enter_context(tc.tile_pool(name="psA", bufs=1, space="PSUM"))

    def ps_tile():
        return psS.tile([P, N], FP, name="s", tag="s")

    # SBUF pools (entered before any single so LIFO holds at ctx exit)
    p_E = ctx.enter_context(tc.tile_pool(name="p_E", bufs=8))
    p_Eb = ctx.enter_context(tc.tile_pool(name="p_Eb", bufs=3))
    p_kqs = ctx.enter_context(tc.tile_pool(name="p_kqs", bufs=6))
    p_den = ctx.enter_context(tc.tile_pool(name="p_den", bufs=8))
    p_out = ctx.enter_context(tc.tile_pool(name="p_out", bufs=3))

    # ------------- singles, bottom of stack = longest-lived -------------
    M_all = single([P, H * C], BF, name="M_all")
    M_sb = [M_all[:, ts(h, C)] for h in range(H)]
    bp_sb = single([P, C], FP, name="bp_sb")
    AT_sb = [single([P, N], BF, name=f"AT{h}") for h in range(H)]
    # natkq[j]: [128, 2N] cols 0:N = k natural (t-major 128-col blocks),
    # N:2N = q natural; features c of heads 2j, 2j+1.
    natkq = [single([P, 2 * N], BF, name=f"natkq{j}") for j in range(CT)]
    qT_sb = [single([P, N], BF, name=f"qT{j}") for j in range(CT)]
    kT_sb = [single([P, N], BF, name=f"kT{j}") for j in range(CT)]
    wqkT_all = single([P, CT * 2 * C], BF, name="wqkT_all")
    wqkT_sb = [wqkT_all[:, ts(i, 2 * C)] for i in range(CT)]
    xT_all = single([P, CT * N], BF, name="xT_all")
    xT_sb = [xT_all[:, ts(i, N)] for i in range(CT)]

    # DRAM scratch for the qT/kT -> natural-layout xbar transposes
    qkTd = []
    for j in range(CT):
        t_, _free = tc.tile([2, P, N], BF, space="DRAM", name=f"qkTd{j}")
        qkTd.append(t_)

    # HAM keep-warm scratch: dummy matmuls hold the PE at 2.4 GHz
    # through the input-DMA window.
    warm_sb = single([P, 512], BF, name="warm_sb")
    nc.gpsimd.memset(warm_sb[:], 0)

    def dummy_mms(n):
        ps = ps_tile()
        for i in range(n):
            nc.tensor.matmul(ps[:, 0:512], lhsT=warm_sb[:, 0:P],
                             rhs=warm_sb[:], start=(i == 0), stop=(i == n - 1))

    # ---------------- batched input DMAs (phase-A inputs first) ---------
    for k in range(CT):
        nc.sync.dma_start(xT_sb[k][:], xT[ts(k, P), :])
        nc.sync.dma_start(wqkT_sb[k][:], wqkT[ts(k, P), :])
    nc.sync.dma_start(M_all[:], M[:])
    nc.sync.dma_start(bp_sb[:], bpr[:])

    # ---------------- projection chains ----------------
    def chain(dst_ap, lhsT_of, rhs_of, width):
        ps = ps_tile()
        for k in range(CT):
            nc.tensor.matmul(
                ps[:, 0:width],
                lhsT=lhsT_of(k),
                rhs=rhs_of(k),
                start=(k == 0),
                stop=(k == CT - 1),
            )
        nc.vector.tensor_copy(dst_ap, ps[:, 0:width])

    def qkT_chains(j):
        def one(which, ch):
            cols = slice(512 * ch, 512 * ch + 512)
            dst = (qT_sb if which == 0 else kT_sb)[j][:, cols]
            woff = C * which
            chain(dst,
                  lambda k: wqkT_sb[k][:, woff + 128 * j: woff + 128 * j + P],
                  lambda k: xT_sb[k][:, cols], 512)
        return [lambda w=w, c=c: one(w, c) for w, c in
                [(1, 0), (0, 0), (1, 1), (0, 1)]]

    def emit_nat_dma(j):
        """qT/kT[j] -> DRAM -> xbar-transposed natural layout natkq[j]."""
        nc.sync.dma_start(qkTd[j][1], kT_sb[j][:])
        nc.sync.dma_start(qkTd[j][0], qT_sb[j][:])
        nc.sync.dma_start_transpose(
            natkq[j][:, 0:N].rearrange("p (t c) -> p t c", c=P),
            qkTd[j][1].rearrange("c (t q) -> c t q", q=P))
        nc.sync.dma_start_transpose(
            natkq[j][:, N:2 * N].rearrange("p (t c) -> p t c", c=P),
            qkTd[j][0].rearrange("c (t q) -> c t q", q=P))

    # warm the PE during the input-DMA window, then qT/kT for pair 0 up
    # front so scores/exp start as early as possible
    for _ in range(3):
        dummy_mms(8)
    for th in qkT_chains(0):
        th()
    emit_nat_dma(0)

    # ---------------- phase B: 6 pair-phases ----------------------------
    at_queue = []

    def drain_at(n):
        while len(at_queue) > n:
            at_queue.pop(0)()

    # chain fragments for pair j+1, spread across pair j's t-steps
    def chain_frags(j, which, ch):
        cols = slice(512 * ch, 512 * ch + 512)
        dst = (qT_sb if which == 0 else kT_sb)[j][:, cols]
        woff = C * which
        box = {}

        def f1():
            ps = ps_tile()
            box["ps"] = ps
            for k in range(3):
                nc.tensor.matmul(
                    ps[:, 0:512],
                    lhsT=wqkT_sb[k][:, woff + 128 * j: woff + 128 * j + P],
                    rhs=xT_sb[k][:, cols], start=(k == 0), stop=False)

        def f2():
            ps = box["ps"]
            for k in range(3, 6):
                nc.tensor.matmul(
                    ps[:, 0:512],
                    lhsT=wqkT_sb[k][:, woff + 128 * j: woff + 128 * j + P],
                    rhs=xT_sb[k][:, cols], start=False, stop=(k == 5))
            nc.vector.tensor_copy(dst, ps[:, 0:512])

        return f1, f2

    # drain targets per t-step: previous pair leaves 10 closures
    DRAIN_TGT = [9, 8, 6, 5, 4, 2, 1, 0]

    for pj in range(NP):
        last_pair = pj == NP - 1
        qt, kt = qT_sb[pj], kT_sb[pj]
        nat3 = natkq[pj].rearrange("p (g t c) -> p g t c", g=2, c=P)
        den = [p_den.tile([P, NT], FP, name=f"den{par}") for par in range(2)]
        rv = [p_den.tile([P, NT], FP, name=f"rv{par}") for par in range(2)]
        Ep = [[None] * NU for _ in range(2)]
        at_box = [{}, {}]

        def at_u(par, u, pj=pj, nat3=nat3, Ep=Ep, rv=rv, at_box=at_box):
            dve = (par, u) in DVE_U

            def work():
                box = at_box[par]
                if "ps" not in box:
                    box["ps"] = psA.tile([P, N], FP, name="at", tag="at")
                at_ps = box["ps"]
                kqs = p_kqs.tile([P, 2, 2 * Z], BF if dve else F8, name="kqst")
                for i in range(2):
                    t = 2 * u + i
                    nc.vector.tensor_scalar(
                        kqs[:, i, :].rearrange("p (g z) -> p g z", g=2),
                        nat3[:, :, t, ts(par, Z)],
                        rv[par][:, t:t + 1], float(1 << KQS_SH),
                        op0=mybir.AluOpType.mult,
                        op1=mybir.AluOpType.mult)
                for ch in range(2):
                    cols = slice(512 * ch, 512 * ch + 512)
                    if dve:
                        for i in range(2):
                            nc.tensor.matmul(
                                at_ps[:, cols],
                                lhsT=kqs[:, i, :],
                                rhs=Ep[par][u][:, i, cols],
                                start=(u == 0 and i == 0),
                                stop=(u == NU - 1 and i == 1),
                            )
                    else:
                        nc.tensor.matmul(
                            at_ps[:, cols],
                            lhsT=kqs[:, 0:2, :],
                            rhs=Ep[par][u][:, 0:2, cols],
                            start=(u == 0), stop=(u == NU - 1),
                            perf_mode=DR,
                        )
                if u == NU - 1:
                    h = 2 * pj + par
                    nc.scalar.mul(AT_sb[h][:], at_ps[:],
                                  2.0 ** (-KQS_SH))
                    at_box[par] = {}
            return work

        # extras: proj chains + nat DMA for pair pj+1
        ext = []
        if not last_pair:
            for which, ch in [(1, 0), (0, 0), (1, 1), (0, 1)]:
                ext.extend(chain_frags(pj + 1, which, ch))
            ext.append(lambda j=pj: emit_nat_dma(j + 1))

        for t in range(NT):
            u, i = t // 2, t & 1
            S2 = []
            for par in range(2):
                S = ps_tile()
                S2.append(S)
                if i == 0:
                    dve = (par, u) in DVE_U
                    Ep[par][u] = (p_Eb.tile([P, 2, N], BF, name="Eb")
                                  if dve else p_E.tile([P, 2, N], F8, name="Ep"))
            # interleaved score MMs: the two heads target PE row groups
            # (0,0) / (64,0) (auto-derived from base_partition) and run
            # concurrently in the array
            for ch in range(2):
                cols = slice(512 * ch, 512 * ch + 512)
                for par in range(2):
                    base = Z * par
                    nc.tensor.matmul(
                        S2[par][:, cols],
                        lhsT=qt[base:base + Z, ts(t, P)],
                        rhs=kt[base:base + Z, cols],
                        start=True, stop=True,
                    )
            for par in range(2):
                E_ap = Ep[par][u][:, i, :]
                if (par, u) in DVE_U:
                    # bf16 Schraudolph bit-trick + DVE row-sum
                    nc.vector.tensor_scalar(
                        E_ap.bitcast(I16), S2[par][:], EXP_K1, EXP_K2,
                        op0=mybir.AluOpType.mult, op1=mybir.AluOpType.add)
                    nc.vector.tensor_reduce(
                        den[par][:, t:t + 1], E_ap,
                        axis=mybir.AxisListType.X, op=mybir.AluOpType.add)
                else:
                    nc.scalar.activation(
                        E_ap, S2[par][:], mybir.ActivationFunctionType.Exp,
                        scale=SCALE, accum_out=den[par][:, t:t + 1],
                    )
            if last_pair:
                # per-u reciprocals + immediate drain: the tail after the
                # last exp shrinks to one u-group + copies + phase C
                if i == 1:
                    for par in range(2):
                        nc.vector.reciprocal(rv[par][:, 2 * u:2 * u + 2],
                                             den[par][:, 2 * u:2 * u + 2])
                        at_queue.append(at_u(par, u))
                drain_at(2 if t < NT - 1 else 0)
            else:
                drain_at(DRAIN_TGT[t])
            for _ in range(2):
                if ext:
                    ext.pop(0)()

        if not last_pair:
            for par in range(2):
                nc.vector.reciprocal(rv[par][:], den[par][:])
            for u in range(NU):
                for par in range(2):
                    at_queue.append(at_u(par, u))
    drain_at(0)

    free_through("natkq0")  # frees xT, wqkT, kT*, qT*, natkq*, warm_sb

    # ---------------- phase C: fused combine + projection + bias ------
    for t in range(NT):
        F_ps = ps_tile()
        for h in range(H):
            for off, w in CCH:
                nc.tensor.matmul(
                    F_ps[:, off:off + w],
                    lhsT=AT_sb[h][:, ts(t, P)],
                    rhs=M_sb[h][:, off:off + w],
                    start=(h == 0), stop=(h == H - 1),
                )
        o = p_out.tile([P, C], FP, name="outt")
        nc.vector.tensor_add(o[:], F_ps[:, 0:C], bp_sb[:])
        nc.sync.dma_start(out[ts(t, P), :], o[:])

    while stack:
        stack.pop()[1]()


def build():
    nc = bacc.Bacc("TRN2", target_bir_lowering=False, debug=False, num_devices=B)
    xT = nc.dram_tensor("xT", [C, N], BF, kind="ExternalInput").ap()
    wqkT = nc.dram_tensor("wqkT", [C, 2 * C], BF, kind="ExternalInput").ap()
    M = nc.dram_tensor("M", [P, H * C], BF, kind="ExternalInput").ap()
    bpr = nc.dram_tensor("bpr", [P, C], FP, kind="ExternalInput").ap()
    out = nc.dram_tensor("out", [N, C], FP, kind="ExternalOutput").ap()
    with tile.TileContext(nc) as tc, ExitStack() as ctx:
        emit(ctx, tc, (xT, wqkT, M, bpr, out))
    nc.compile()
    return nc


def kernel(x, Wq, Wk, Wp, bp, trace=False, **trace_kwargs):
    global last_results
    x = np.asarray(x, dtype=np.float32)
    Wq = np.asarray(Wq, dtype=np.float32)
    Wk = np.asarray(Wk, dtype=np.float32)
    Wp = np.asarray(Wp, dtype=np.float32)
    bp = np.asarray(bp, dtype=np.float32)

    nc = build()
    bf = ml_dtypes.bfloat16
    wqkTc = np.ascontiguousarray(
        np.concatenate([Wq.T, Wk.T], axis=1)).astype(bf)  # [C, 2C]
    # fused combine+projection weights: M_hT = [Wq_h; Wk_h] @ Wp^T  [2Z, C]
    Wq_h = Wq.reshape(H, Z, C)
    Wk_h = Wk.reshape(H, Z, C)
    W2 = np.concatenate([Wq_h, Wk_h], axis=1)             # [H, 2Z, C]
    M_np = np.einsum("hzc,dc->hzd", W2, Wp)               # [H, 2Z, C]
    Mc = np.ascontiguousarray(
        M_np.transpose(1, 0, 2).reshape(P, H * C)).astype(bf)
    bprc = np.ascontiguousarray(
        np.broadcast_to(bp.reshape(1, C), (P, C)).astype(np.float32))
    in_maps = []
    for b in range(B):
        in_maps.append({
            "xT": np.ascontiguousarray(x[b].T).astype(bf),
            "wqkT": wqkTc, "M": Mc, "bpr": bprc,
        })
    res = bass_utils.run_bass_kernel_spmd(
        nc, in_maps, core_ids=list(range(B)), trace=trace, **trace_kwargs)
    last_results = res
    return np.stack([res.results[b]["out"] for b in range(B)], axis=0)
